# revision 1
# baseline (speedup 1.0000x reference)
"""Trainium2 Bass kernel for nn_CustomAttention (B=16, R=128, D=128, BD=64).

Sharding: Wq (R,R,D,D) is split along the target-region axis s across the
8 cores (16 s-values per core).  Each core computes its slice of
Q/scores/attended; kv_embed and the shared K/V projections are replicated.

Per-core device layout: the 16 local s-values are processed as 4 "quads";
a quad's 4 members occupy 16-row blocks of PSUM at partition bases
0/32/64/96 (PE column-group alignment -- only 32-aligned output bases are
legal), with batch b in the 16 rows of each block.  The gap rows carry
benign garbage that is never read downstream.

The Wq stream (the memory-bound term: 128 MiB/core) is shipped as two
bf16 planes (hi + residual lo) -- same bytes as fp32 -- because PE column
tiling only supports 16-bit matmuls; Q = qh@Wh + qh@Wl + bias recovers
~2e-3 relative accuracy (fp32 PSUM accumulation).  All base-0 matmuls
(K/V projections, attended, output projection) run in float32r.

Scores are reduced on DVE against K replicated 8x over partitions;
attention row-normalization runs on ACT (relu+accum, copy*scale),
V row-normalization on ACT/DVE, attended + output projection on PE.
"""

import numpy as np
import ml_dtypes

try:
    import concourse  # noqa: F401
except ImportError:  # pragma: no cover
    import sys

    sys.path.insert(0, "/opt/trn_rl_repo")

from contextlib import ExitStack

import concourse.mybir as mybir
import concourse.tile as tile
from concourse import bacc
from concourse.bass_utils import run_bass_kernel_spmd
from concourse.masks import make_identity

F32 = mybir.dt.float32
F32R = mybir.dt.float32r
BF16 = mybir.dt.bfloat16
AF = mybir.ActivationFunctionType
ALU = mybir.AluOpType
AXL = mybir.AxisListType
NPBF16 = ml_dtypes.bfloat16

B, R, D, BD = 16, 128, 128, 64
NCORES = 8
SLOC = R // NCORES          # 16 s-values per core
NQ = SLOC // 4              # 4 quads of 4 members
TCH = 8                     # t-values per DMA chunk
NTC = R // TCH              # 16 chunks
GP = TCH // 4               # psum groups (of 4 t) per chunk
CHF = TCH * D               # free elements per chunk (1024)

_CACHE = {}


def _build():
    nc = bacc.Bacc("TRN2", target_bir_lowering=False, debug=False,
                   enable_asserts=True, num_devices=NCORES)

    def dram_in(name, shape, dt):
        return nc.dram_tensor(name, shape, dt, kind="ExternalInput").ap()

    wqh_ap = dram_in("wqh", [SLOC, NTC, D, CHF], BF16)  # [s][tc][i][chunk] hi
    wql_ap = dram_in("wql", [SLOC, NTC, D, CHF], BF16)  # [s][tc][i][chunk] lo
    qt_ap = dram_in("qt", [D, SLOC * B], BF16)          # [i][(s,b)]
    bq_ap = dram_in("bqd", [SLOC, R * D], BF16)         # [s][(t,d)]
    kvt_ap = dram_in("kvt", [D, B * R], F32R)           # [i][(b,t)]
    wk_ap = dram_in("wk", [D, D], F32R)
    wv_ap = dram_in("wv", [D, BD], F32R)
    wr_ap = dram_in("wr", [BD, D], F32R)
    sel_ap = dram_in("sel", [128, SLOC * B], BF16)      # bias row selectors (K=128, zero-padded)
    bk_ap = dram_in("bkr", [R, D], F32)                 # bk tiled over partitions
    bv_ap = dram_in("bvr", [R, BD], F32)
    br_ap = dram_in("brr", [R, D], F32)

    awout_ap = nc.dram_tensor("awout", [4, B, NQ * R], F32,
                              kind="ExternalOutput").ap()
    attout_ap = nc.dram_tensor("attout", [2, 128, D], F32,
                               kind="ExternalOutput").ap()

    with tile.TileContext(nc) as tc:
        with ExitStack() as ctx:
            per = ctx.enter_context(tc.tile_pool(name="persist", bufs=1))
            pre = ctx.enter_context(tc.tile_pool(name="prelude", bufs=2))
            wqp = ctx.enter_context(tc.tile_pool(name="wqpool", bufs=38))
            tmpp = ctx.enter_context(tc.tile_pool(name="tmppool", bufs=3))
            qpsp = ctx.enter_context(tc.tile_pool(name="qps", bufs=5, space="PSUM"))
            aux = ctx.enter_context(tc.tile_pool(name="aux", bufs=2, space="PSUM"))
            aux1 = ctx.enter_context(tc.tile_pool(name="aux1", bufs=1, space="PSUM"))

            # ---- constants / small inputs ----
            qt = per.tile([D, SLOC * B], BF16)
            kvt = per.tile([D, B * R], F32R)
            wk = per.tile([D, D], F32R)
            wv = per.tile([D, BD], F32R)
            wr = per.tile([BD, D], F32R)
            sel = per.tile([128, SLOC * B], BF16)
            bk_rep = per.tile([R, D], F32)
            bv_rep = per.tile([R, BD], F32)
            br_rep = per.tile([R, D], F32)
            ident = per.tile([128, 128], F32)
            for t, ap in ((qt, qt_ap), (kvt, kvt_ap), (wk, wk_ap), (wv, wv_ap),
                          (wr, wr_ap), (sel, sel_ap), (bk_rep, bk_ap),
                          (bv_rep, bv_ap), (br_rep, br_ap)):
                nc.sync.dma_start(t[:], ap[:])
            make_identity(nc, ident[:])

            krep = per.tile([128, R * D], F32)          # K[b] rows, period 16
            vn = per.tile([R, B * BD], F32R)            # V_norm[b] as [t, (b,dd)]

            # bias tile (K=128-padded with zeros) -- loaded first so the
            # in-order gpsimd stream unblocks the tc=0 bias matmuls early
            bq_all = per.tile([128, R * D], BF16)
            nc.gpsimd.memset(bq_all[:], 0.0)
            nc.gpsimd.dma_start(bq_all[0:SLOC, :], bq_ap[:, :])

            # ---- prelude: K = kv@Wk + bk, V_norm from kv@Wv + bv ----
            for b in range(B):
                lhs = kvt[:, b * R:(b + 1) * R]         # [i, t] for this b
                kb_ps = aux.tile([128, D], F32, tag="auxA")
                nc.tensor.matmul(kb_ps[:], lhs, wk[:], start=True, stop=True)
                kb = pre.tile([R, D], F32, tag="kb")
                nc.vector.tensor_add(kb[:], kb_ps[:], bk_rep[:])
                nc.gpsimd.dma_start(
                    krep[b:b + 1, :].rearrange("o (t d) -> o t d", t=R), kb[:])

                vb_ps = aux.tile([128, BD], F32, tag="auxA")
                nc.tensor.matmul(vb_ps[:], lhs, wv[:], start=True, stop=True)
                vsb = pre.tile([R, BD], F32, tag="vsb")
                nc.vector.tensor_add(vsb[:], vb_ps[:], bv_rep[:])
                vsq = pre.tile([R, BD], F32, tag="vsq")
                ss = pre.tile([R, 1], F32, tag="ss")
                nc.scalar.activation(vsq[:], vsb[:], AF.Square, accum_out=ss[:])
                nrm = pre.tile([R, 1], F32, tag="nrm")
                nc.scalar.activation(nrm[:], ss[:], AF.Sqrt)
                nc.vector.tensor_scalar_max(nrm[:], nrm[:], 1e-12)
                vri = pre.tile([R, 1], F32, tag="vri")
                nc.vector.reciprocal(vri[:], nrm[:])
                nc.vector.tensor_scalar_mul(vn[:, b * BD:(b + 1) * BD], vsb[:], vri[:])

            aw_raw = per.tile([128, NQ * R], F32)       # row 32j+b, col q*128+t

            # ---- main loop: stream Wq (hi+lo), Q-projection + scores ----
            for rep in range(7):
                nc.gpsimd.dma_start(krep[16 * (rep + 1):16 * (rep + 2), :],
                                    krep[0:16, :])

            for tcx in range(NTC):
                wts = []
                for s in range(SLOC):
                    wh = wqp.tile([D, CHF], BF16, tag="wqt")
                    nc.sync.dma_start(wh[:], wqh_ap[s, tcx])
                    wl = wqp.tile([D, CHF], BF16, tag="wqt")
                    nc.sync.dma_start(wl[:], wql_ap[s, tcx])
                    wts.append((wh, wl))
                for q in range(NQ):
                    for g in range(GP):
                        gsl = slice(g * 512, (g + 1) * 512)
                        qps = qpsp.tile([128, 512], F32, tag="qps")
                        for j in range(4):
                            s = 4 * q + j
                            out = qps[32 * j:32 * j + B, :]
                            lhsq = qt[:, s * B:(s + 1) * B]
                            nc.tensor.matmul(out, lhsq, wts[s][0][:, gsl],
                                             start=True, stop=False,
                                             tile_position=(0, 32 * j))
                            nc.tensor.matmul(out, lhsq, wts[s][1][:, gsl],
                                             start=False, stop=False,
                                             tile_position=(0, 32 * j))
                            nc.tensor.matmul(out, sel[:, s * B:(s + 1) * B],
                                             bq_all[:, tcx * CHF + g * 512:
                                                    tcx * CHF + (g + 1) * 512],
                                             start=False, stop=True,
                                             tile_position=(0, 32 * j))
                        tmp = tmpp.tile([128, 512], F32, tag="tmp")
                        kslice = krep[:, tcx * CHF + g * 512:tcx * CHF + (g + 1) * 512]
                        nc.vector.tensor_mul(tmp[:], qps[:], kslice)
                        col = q * R + tcx * TCH + g * 4
                        nc.vector.tensor_reduce(
                            aw_raw[:, col:col + 4],
                            tmp[:].rearrange("p (t d) -> p t d", t=4),
                            axis=AXL.X, op=ALU.add)

            # ---- postlude: relu, normalize, attended, output proj ----
            aw_norm = per.tile([128, NQ * R], F32)
            ssum = per.tile([128, NQ], F32)
            rden = per.tile([128, NQ], F32)
            rinv = per.tile([128, NQ], F32)
            awt = per.tile([128, NQ * R], F32R)

            for q in range(NQ):
                nc.scalar.activation(aw_norm[:, q * R:(q + 1) * R],
                                     aw_raw[:, q * R:(q + 1) * R], AF.Relu,
                                     accum_out=ssum[:, q:q + 1])
            nc.vector.tensor_scalar_add(rden[:], ssum[:], 1e-8)
            nc.vector.reciprocal(rinv[:], rden[:])
            for q in range(NQ):
                nc.scalar.activation(aw_norm[:, q * R:(q + 1) * R],
                                     aw_norm[:, q * R:(q + 1) * R], AF.Copy,
                                     scale=rinv[:, q:q + 1])
            for j in range(4):
                nc.sync.dma_start(awout_ap[j], aw_norm[32 * j:32 * j + B, :])
            for q in range(NQ):
                trp = aux.tile([128, 128], F32, tag="auxA")
                nc.tensor.transpose(trp[:], aw_norm[:, q * R:(q + 1) * R], ident[:])
                nc.vector.tensor_copy(awt[:, q * R:(q + 1) * R], trp[:])

            attps = aux1.tile([BD, B * SLOC], F32, tag="auxC")
            awt4 = awt[:].rearrange("p (q j c) -> p q j c", q=NQ, j=4, c=32)
            for b in range(B):
                nc.tensor.matmul(attps[:, b * SLOC:(b + 1) * SLOC],
                                 vn[:, b * BD:(b + 1) * BD], awt4[:, :, :, b],
                                 start=True, stop=True)
            atts = per.tile([BD, B * SLOC], F32R)
            nc.vector.tensor_copy(atts[:], attps[:])

            for h in range(2):
                finps = aux.tile([128, D], F32, tag="auxA")
                nc.tensor.matmul(finps[:], atts[:, h * 128:(h + 1) * 128], wr[:],
                                 start=True, stop=True)
                fin = per.tile([128, D], F32, tag=f"fin{h}")
                nc.vector.tensor_add(fin[:], finps[:], br_rep[:])
                nc.sync.dma_start(attout_ap[h], fin[:])

    nc.compile()
    return nc


def _get_nc():
    if "nc" not in _CACHE:
        _CACHE["nc"] = _build()
    return _CACHE["nc"]


def _make_in_maps(q_embed, kv_embed, Wk, bk, Wv, bv, Wq, bq, Wr, br):
    sel = np.zeros((128, SLOC * B), NPBF16)
    for s in range(SLOC):
        sel[s, s * B:(s + 1) * B] = 1.0
    shared = {
        "kvt": np.ascontiguousarray(
            kv_embed.transpose(2, 0, 1).reshape(D, B * R)),
        "wk": np.ascontiguousarray(Wk),
        "wv": np.ascontiguousarray(Wv),
        "wr": np.ascontiguousarray(Wr),
        "sel": sel,
        "bkr": np.ascontiguousarray(np.broadcast_to(bk, (R, D))),
        "bvr": np.ascontiguousarray(np.broadcast_to(bv, (R, BD))),
        "brr": np.ascontiguousarray(np.broadcast_to(br, (R, D))),
    }
    in_maps = []
    for c in range(NCORES):
        s0 = SLOC * c
        m = dict(shared)
        wq_c = np.ascontiguousarray(
            Wq[s0:s0 + SLOC].transpose(0, 2, 1, 3).reshape(
                SLOC, D, NTC, CHF).transpose(0, 2, 1, 3))
        wq_hi = wq_c.astype(NPBF16)
        m["wqh"] = wq_hi
        m["wql"] = (wq_c - wq_hi.astype(np.float32)).astype(NPBF16)
        m["qt"] = np.ascontiguousarray(
            q_embed[:, s0:s0 + SLOC, :].transpose(2, 1, 0)).reshape(
                D, SLOC * B).astype(NPBF16)
        m["bqd"] = np.ascontiguousarray(
            bq[s0:s0 + SLOC]).reshape(SLOC, R * D).astype(NPBF16)
        in_maps.append(m)
    return in_maps


def _assemble(results):
    attended = np.empty((B, R, D), np.float32)
    aw = np.empty((B, R, R), np.float32)
    for c, r in enumerate(results):
        s0 = SLOC * c
        att = r["attout"].reshape(B, SLOC, D)
        attended[:, s0:s0 + SLOC, :] = att
        a = r["awout"].reshape(4, B, NQ, R).transpose(1, 2, 0, 3).reshape(B, SLOC, R)
        aw[:, s0:s0 + SLOC, :] = a
    return attended, aw


def _execute(inputs, **kwargs):
    nc = _get_nc()
    args = {k: np.asarray(inputs[k], np.float32) for k in
            ("q_embed", "kv_embed", "Wk", "bk", "Wv", "bv", "Wq", "bq",
             "Wr", "br")}
    in_maps = _make_in_maps(**args)
    res = run_bass_kernel_spmd(nc, in_maps, core_ids=list(range(NCORES)),
                               **kwargs)
    return _assemble(res.results), res


def kernel(**inputs):
    (attended, aw), _ = _execute(inputs)
    return attended, aw



# revision 9
# speedup vs baseline: 1.7091x; 1.7091x over previous
"""Trainium2 Bass kernel for nn_CustomAttention (B=16, R=128, D=128, BD=64).

Sharding: Wq (R,R,D,D) is split along the target-region axis s across the
8 cores (16 s-values per core).  Each core computes its slice of
Q/scores/attended; kv_embed and the shared K/V projections are replicated.

v2 layout vs v1:
 * Wq ships as a SINGLE bf16 plane (64 MiB/core, half of v1's hi+lo pair).
   Verified numerics: aw rel err ~2.7e-3 against the 2e-2 gate.
 * The query bias bq no longer rides the PE stream.  Its score contribution
   bqK[b,s,t] = bq[s,t,:].K[b,t,:] is precomputed with 512 tiny matmuls
   against K^T and folded into the score reduction as the initial value of
   tensor_tensor_reduce -- removing a third of the PE column stream.
 * Wq arrives in 2 MiB DMA transfers (8 s-values per transfer, 2 per
   chunk) instead of 16x256 KiB -- fewer, larger descriptors.
 * K is replicated across partition groups (krep) in bf16, halving the
   SBUF-to-SBUF replication traffic; replication uses 3 doubling copies.
 * Scores use tensor_mul + tensor_reduce on DVE (tensor_tensor_reduce
   miscompiles for hardware; verified broken in bisect) with one
   tensor_add folding in the bqK bias term after the stream.

Per-core device layout: the 16 local s-values are processed as 4 "quads";
a quad's 4 members occupy 16-row blocks of PSUM at partition bases
0/32/64/96 (PE column-group alignment), with batch b in the 16 rows of
each block.  Gap rows carry benign garbage that is never read downstream.
"""

import numpy as np
import ml_dtypes

try:
    import concourse  # noqa: F401
except ImportError:  # pragma: no cover
    import sys

    sys.path.insert(0, "/opt/trn_rl_repo")

from contextlib import ExitStack

import concourse.mybir as mybir
import concourse.tile as tile
from concourse import bacc
from concourse.bass_utils import run_bass_kernel_spmd
from concourse.masks import make_identity

F32 = mybir.dt.float32
F32R = mybir.dt.float32r
BF16 = mybir.dt.bfloat16
AF = mybir.ActivationFunctionType
ALU = mybir.AluOpType
AXL = mybir.AxisListType
NPBF16 = ml_dtypes.bfloat16

B, R, D, BD = 16, 128, 128, 64
NCORES = 8
SLOC = R // NCORES          # 16 s-values per core
NQ = SLOC // 4              # 4 quads of 4 members
TCH = 8                     # t-values per chunk
NTC = R // TCH              # 16 chunks
GP = TCH // 4               # psum groups (of 4 t) per chunk
CHF = TCH * D               # free elements per chunk per s (1024)
HHF = 8 * CHF               # free elements per half-chunk DMA (8 s)

_CACHE = {}


def _build():
    nc = bacc.Bacc("TRN2", target_bir_lowering=False, debug=False,
                   enable_asserts=True, num_devices=NCORES)

    def dram_in(name, shape, dt):
        return nc.dram_tensor(name, shape, dt, kind="ExternalInput").ap()

    wq_ap = dram_in("wq", [NTC, 2, D, HHF], BF16)   # [tc][half][i][(s8,t8,d)]
    qt_ap = dram_in("qt", [D, SLOC * B], BF16)      # [i][(s,b)]
    bqt_ap = dram_in("bqt", [D, R * SLOC], BF16)    # [d][(t,j,q)]
    kvt_ap = dram_in("kvt", [D, B * R], F32R)       # [i][(b,t)]
    wk_ap = dram_in("wk", [D, D], F32R)
    wv_ap = dram_in("wv", [D, BD], F32R)
    wr_ap = dram_in("wr", [BD, D], F32R)
    bk_ap = dram_in("bkr", [R, D], F32)             # bk tiled over partitions
    bv_ap = dram_in("bvr", [R, BD], F32)
    br_ap = dram_in("brr", [R, D], F32)

    awout_ap = nc.dram_tensor("awout", [4, B, NQ * R], F32,
                              kind="ExternalOutput").ap()
    attout_ap = nc.dram_tensor("attout", [2, 128, D], F32,
                               kind="ExternalOutput").ap()

    with tile.TileContext(nc) as tc:
        with ExitStack() as ctx:
            per = ctx.enter_context(tc.tile_pool(name="persist", bufs=1))
            pre = ctx.enter_context(tc.tile_pool(name="prelude", bufs=2))
            wqp = ctx.enter_context(tc.tile_pool(name="wqpool", bufs=8))
            scrp = ctx.enter_context(tc.tile_pool(name="scrpool", bufs=2))
            qpsp = ctx.enter_context(tc.tile_pool(name="qps", bufs=4, space="PSUM"))
            aux = ctx.enter_context(tc.tile_pool(name="aux", bufs=2, space="PSUM"))
            aux1 = ctx.enter_context(tc.tile_pool(name="aux1", bufs=1, space="PSUM"))
            bqkp = ctx.enter_context(tc.tile_pool(name="bqkp", bufs=1, space="PSUM"))

            # ---- constants / small inputs ----
            # qt and kts carry 16 columns of zero padding so the 32-wide
            # stationary slices below stay in bounds for the last s / t;
            # the extra output rows land in otherwise-unused psum gap rows.
            qt = per.tile([D, SLOC * B + B], BF16)
            kvt = per.tile([D, B * R], F32R)
            wk = per.tile([D, D], F32R)
            bqt = per.tile([D, R * SLOC], BF16)
            wv = per.tile([D, BD], F32R)
            wr = per.tile([BD, D], F32R)
            bk_rep = per.tile([R, D], F32)
            bv_rep = per.tile([R, BD], F32)
            br_rep = per.tile([R, D], F32)
            ident = per.tile([128, 128], F32)
            # critical-path inputs on the sync (HWDGE) queue, ahead of wq
            nc.gpsimd.memset(qt[:, SLOC * B:], 0.0)
            for t, ap in ((kvt, kvt_ap), (wk, wk_ap), (bqt, bqt_ap)):
                nc.sync.dma_start(t[:], ap[:])
            nc.sync.dma_start(qt[:, :SLOC * B], qt_ap[:])
            # the rest on the gpsimd (SWDGE) queue
            for t, ap in ((wv, wv_ap), (wr, wr_ap), (bk_rep, bk_ap),
                          (bv_rep, bv_ap), (br_rep, br_ap)):
                nc.gpsimd.dma_start(t[:], ap[:])
            make_identity(nc, ident[:])

            krep = per.tile([128, R * D], BF16)     # K[b] rows, period 16
            kts = per.tile([D, R * B + B], BF16)    # K^T as [d, (t,b)], padded
            nc.gpsimd.memset(kts[:, R * B:], 0.0)
            vn = per.tile([R, B * BD], F32R)        # V_norm[b] as [t, (b,dd)]
            bqk = per.tile([128, NQ * R], F32)      # bias scores, aw layout

            # ---- K = kv@Wk + bk  (rows t), K^T, krep ----
            for b in range(B):
                lhs = kvt[:, b * R:(b + 1) * R]     # [i, t] for this b
                kb_ps = aux.tile([128, D], F32, tag="auxA")
                nc.tensor.matmul(kb_ps[:], lhs, wk[:], start=True, stop=True)
                kb = pre.tile([R, D], F32, tag="kb")
                nc.vector.tensor_add(kb[:], kb_ps[:], bk_rep[:])
                kbf = pre.tile([R, D], BF16, tag="kbf")
                nc.vector.tensor_copy(kbf[:], kb[:])
                nc.gpsimd.dma_start(
                    krep[b:b + 1, :].rearrange("o (t d) -> o t d", t=R), kbf[:])
                # K^T columns (t, b): strided DVE copy from a PE transpose
                tr_ps = aux.tile([128, D], F32, tag="auxA")
                nc.tensor.transpose(tr_ps[:], kb[:], ident[:])
                nc.vector.tensor_copy(
                    kts[:, :R * B].rearrange("p (t b) -> p t b", b=B)[:, :, b],
                    tr_ps[:])
            # replicate krep rows 0:16 over all 8 groups (doubling)
            nc.gpsimd.dma_start(krep[16:32, :], krep[0:16, :])
            nc.gpsimd.dma_start(krep[32:64, :], krep[0:32, :])
            nc.gpsimd.dma_start(krep[64:128, :], krep[0:64, :])

            # ---- bqK[32j+b, (t,q)] = bq[4q+j,t,:].K[b,t,:] ----
            bqk_ps = bqkp.tile([128, NQ * R], F32, tag="bqk")
            for t in range(R):
                kt_sl = kts[:, t * B:t * B + 2 * B]  # [d, 32] stationary
                for j in range(4):
                    nc.tensor.matmul(
                        bqk_ps[32 * j:32 * (j + 1), t * NQ:(t + 1) * NQ],
                        kt_sl, bqt[:, t * SLOC + j * NQ:t * SLOC + (j + 1) * NQ],
                        start=True, stop=True, tile_position=(0, 32 * j))
            nc.vector.tensor_copy(
                bqk[:].rearrange("p (q t) -> p t q", q=NQ),
                bqk_ps[:].rearrange("p (t q) -> p t q", q=NQ))

            # ---- V_norm from kv@Wv + bv ----
            for b in range(B):
                lhs = kvt[:, b * R:(b + 1) * R]
                vb_ps = aux.tile([128, BD], F32, tag="auxA")
                nc.tensor.matmul(vb_ps[:], lhs, wv[:], start=True, stop=True)
                vsb = pre.tile([R, BD], F32, tag="vsb")
                nc.vector.tensor_add(vsb[:], vb_ps[:], bv_rep[:])
                vsq = pre.tile([R, BD], F32, tag="vsq")
                ss = pre.tile([R, 1], F32, tag="ss")
                nc.scalar.activation(vsq[:], vsb[:], AF.Square, accum_out=ss[:])
                nrm = pre.tile([R, 1], F32, tag="nrm")
                nc.scalar.activation(nrm[:], ss[:], AF.Sqrt)
                nc.vector.tensor_scalar_max(nrm[:], nrm[:], 1e-12)
                vri = pre.tile([R, 1], F32, tag="vri")
                nc.vector.reciprocal(vri[:], nrm[:])
                nc.vector.tensor_scalar_mul(vn[:, b * BD:(b + 1) * BD], vsb[:], vri[:])

            aw_raw = per.tile([128, NQ * R], F32)   # row 32j+b, col q*128+t

            # ---- main loop: stream Wq, Q-projection + fused scores ----
            for tcx in range(NTC):
                halves = []
                for h in range(2):
                    wt = wqp.tile([D, HHF], BF16, tag="wqt")
                    nc.sync.dma_start(wt[:], wq_ap[tcx, h])
                    halves.append(wt)
                for q in range(NQ):
                    wt = halves[q // 2]
                    for g in range(GP):
                        qps = qpsp.tile([128, 512], F32, tag="qps")
                        for j in range(4):
                            s = 4 * q + j
                            s8 = s % 8
                            nc.tensor.matmul(
                                qps[32 * j:32 * (j + 1), :],
                                qt[:, s * B:s * B + 2 * B],
                                wt[:, s8 * CHF + g * 512:s8 * CHF + (g + 1) * 512],
                                start=True, stop=True,
                                tile_position=(0, 32 * j))
                        t0 = tcx * TCH + g * 4
                        col = q * R + t0
                        tmp = scrp.tile([128, 512], F32, tag="tmp")
                        nc.vector.tensor_mul(tmp[:], qps[:],
                                             krep[:, t0 * D:(t0 + 4) * D])
                        nc.vector.tensor_reduce(
                            aw_raw[:, col:col + 4],
                            tmp[:].rearrange("p (t d) -> p t d", t=4),
                            axis=AXL.X, op=ALU.add)

            # ---- postlude: relu, normalize, attended, output proj ----
            aw_norm = per.tile([128, NQ * R], F32)
            ssum = per.tile([128, NQ], F32)
            rden = per.tile([128, NQ], F32)
            rinv = per.tile([128, NQ], F32)
            awt = per.tile([128, NQ * R], F32R)

            nc.vector.tensor_add(aw_raw[:], aw_raw[:], bqk[:])
            for q in range(NQ):
                nc.scalar.activation(aw_norm[:, q * R:(q + 1) * R],
                                     aw_raw[:, q * R:(q + 1) * R], AF.Relu,
                                     accum_out=ssum[:, q:q + 1])
            nc.vector.tensor_scalar_add(rden[:], ssum[:], 1e-8)
            nc.vector.reciprocal(rinv[:], rden[:])
            for q in range(NQ):
                nc.scalar.activation(aw_norm[:, q * R:(q + 1) * R],
                                     aw_norm[:, q * R:(q + 1) * R], AF.Copy,
                                     scale=rinv[:, q:q + 1])
            for j in range(4):
                nc.gpsimd.dma_start(awout_ap[j], aw_norm[32 * j:32 * j + B, :])
            for q in range(NQ):
                trp = aux.tile([128, 128], F32, tag="auxA")
                nc.tensor.transpose(trp[:], aw_norm[:, q * R:(q + 1) * R], ident[:])
                nc.vector.tensor_copy(awt[:, q * R:(q + 1) * R], trp[:])

            attps = aux1.tile([BD, B * SLOC], F32, tag="auxC")
            awt4 = awt[:].rearrange("p (q j c) -> p q j c", q=NQ, j=4, c=32)
            for b in range(B):
                nc.tensor.matmul(attps[:, b * SLOC:(b + 1) * SLOC],
                                 vn[:, b * BD:(b + 1) * BD], awt4[:, :, :, b],
                                 start=True, stop=True)
            atts = per.tile([BD, B * SLOC], F32R)
            nc.vector.tensor_copy(atts[:], attps[:])

            for h in range(2):
                finps = aux.tile([128, D], F32, tag="auxA")
                nc.tensor.matmul(finps[:], atts[:, h * 128:(h + 1) * 128], wr[:],
                                 start=True, stop=True)
                fin = per.tile([128, D], F32, tag=f"fin{h}")
                nc.vector.tensor_add(fin[:], finps[:], br_rep[:])
                nc.gpsimd.dma_start(attout_ap[h], fin[:])

    nc.compile()
    return nc


def _get_nc():
    if "nc" not in _CACHE:
        _CACHE["nc"] = _build()
    return _CACHE["nc"]


def _make_in_maps(q_embed, kv_embed, Wk, bk, Wv, bv, Wq, bq, Wr, br):
    shared = {
        "kvt": np.ascontiguousarray(
            kv_embed.transpose(2, 0, 1).reshape(D, B * R)),
        "wk": np.ascontiguousarray(Wk),
        "wv": np.ascontiguousarray(Wv),
        "wr": np.ascontiguousarray(Wr),
        "bkr": np.ascontiguousarray(np.broadcast_to(bk, (R, D))),
        "bvr": np.ascontiguousarray(np.broadcast_to(bv, (R, BD))),
        "brr": np.ascontiguousarray(np.broadcast_to(br, (R, D))),
    }
    in_maps = []
    for c in range(NCORES):
        s0 = SLOC * c
        m = dict(shared)
        # [s, t, i, d] -> [tc, h, i, s8, t8, d] with s = 8h+s8, t = 8tc+t8
        wq_c = Wq[s0:s0 + SLOC].reshape(2, 8, NTC, TCH, D, D)
        m["wq"] = np.ascontiguousarray(
            wq_c.transpose(2, 0, 4, 1, 3, 5)).reshape(
                NTC, 2, D, HHF).astype(NPBF16)
        m["qt"] = np.ascontiguousarray(
            q_embed[:, s0:s0 + SLOC, :].transpose(2, 1, 0)).reshape(
                D, SLOC * B).astype(NPBF16)
        # [s, t, d] with s = 4q+j -> [d, (t, j, q)]
        bq_c = bq[s0:s0 + SLOC].reshape(NQ, 4, R, D)
        m["bqt"] = np.ascontiguousarray(
            bq_c.transpose(3, 2, 1, 0)).reshape(D, R * SLOC).astype(NPBF16)
        in_maps.append(m)
    return in_maps


def _assemble(results):
    attended = np.empty((B, R, D), np.float32)
    aw = np.empty((B, R, R), np.float32)
    for c, r in enumerate(results):
        s0 = SLOC * c
        att = r["attout"].reshape(B, SLOC, D)
        attended[:, s0:s0 + SLOC, :] = att
        a = r["awout"].reshape(4, B, NQ, R).transpose(1, 2, 0, 3).reshape(B, SLOC, R)
        aw[:, s0:s0 + SLOC, :] = a
    return attended, aw


def _execute(inputs, **kwargs):
    nc = _get_nc()
    args = {k: np.asarray(inputs[k], np.float32) for k in
            ("q_embed", "kv_embed", "Wk", "bk", "Wv", "bv", "Wq", "bq",
             "Wr", "br")}
    in_maps = _make_in_maps(**args)
    res = run_bass_kernel_spmd(nc, in_maps, core_ids=list(range(NCORES)),
                               **kwargs)
    return _assemble(res.results), res


def kernel(**inputs):
    (attended, aw), _ = _execute(inputs)
    return attended, aw


# revision 15
# speedup vs baseline: 2.5374x; 1.4847x over previous
"""Trainium2 Bass kernel for nn_CustomAttention (B=16, R=128, D=128, BD=64).

Sharding: Wq (R,R,D,D) is split along the target-region axis s across the
8 cores (16 s-values per core).  Each core computes its slice of
Q/scores/attended; kv_embed and the shared K/V projections are replicated.

v2 layout vs v1:
 * Wq ships as a SINGLE bf16 plane (64 MiB/core, half of v1's hi+lo pair).
   Verified numerics: aw rel err ~2.7e-3 against the 2e-2 gate.
 * The query bias bq no longer rides the PE stream.  Its score contribution
   bqK[b,s,t] = bq[s,t,:].K[b,t,:] is precomputed with 512 tiny matmuls
   against K^T and folded into the score reduction as the initial value of
   tensor_tensor_reduce -- removing a third of the PE column stream.
 * Wq arrives as one 4.25 MiB DMA per chunk, alternating between the two
   HWDGE rings (sync / scalar) so each ring's ~2us completion bubble is
   hidden under the other ring's data movement.
 * K is replicated across partition groups (krep) in bf16, halving the
   SBUF-to-SBUF replication traffic; replication uses 3 doubling copies.
 * Scores use tensor_mul + tensor_reduce on DVE (tensor_tensor_reduce
   miscompiles for hardware; verified broken in bisect) with one
   tensor_add folding in the bqK bias term after the stream.

Per-core device layout: the 16 local s-values are processed as 4 "quads";
a quad's 4 members occupy 16-row blocks of PSUM at partition bases
0/32/64/96 (PE column-group alignment), with batch b in the 16 rows of
each block.  Gap rows carry benign garbage that is never read downstream.
"""

import numpy as np
import ml_dtypes

try:
    import concourse  # noqa: F401
except ImportError:  # pragma: no cover
    import sys

    sys.path.insert(0, "/opt/trn_rl_repo")

from contextlib import ExitStack

import concourse.mybir as mybir
import concourse.tile as tile
from concourse import bacc
from concourse.bass_utils import run_bass_kernel_spmd
from concourse.masks import make_identity

F32 = mybir.dt.float32
F32R = mybir.dt.float32r
BF16 = mybir.dt.bfloat16
AF = mybir.ActivationFunctionType
ALU = mybir.AluOpType
AXL = mybir.AxisListType
NPBF16 = ml_dtypes.bfloat16

B, R, D, BD = 16, 128, 128, 64
NCORES = 8
SLOC = R // NCORES          # 16 s-values per core
NQ = SLOC // 4              # 4 quads of 4 members
TCH = 8                     # t-values per chunk
NTC = R // TCH              # 16 chunks
GP = TCH // 4               # psum groups (of 4 t) per chunk
CHF = TCH * D               # free elements per chunk per s (1024)
HHF = 8 * CHF               # free elements per half-chunk DMA (8 s)

_CACHE = {}


def _build():
    nc = bacc.Bacc("TRN2", target_bir_lowering=False, debug=False,
                   enable_asserts=True, num_devices=NCORES)

    def dram_in(name, shape, dt):
        return nc.dram_tensor(name, shape, dt, kind="ExternalInput").ap()

    wq_ap = dram_in("wq", [NTC, D, SLOC * CHF], BF16)  # [tc][i][(s,t8,d)]
    qt_ap = dram_in("qt", [D, SLOC * B], BF16)      # [i][(s,b)]
    bqt_ap = dram_in("bqt", [D, R * SLOC], BF16)    # [d][(t,j,q)]
    kvt_ap = dram_in("kvt", [D, B * R], F32R)       # [i][(b,t)]
    wk_ap = dram_in("wk", [D, D], F32R)
    wv_ap = dram_in("wv", [D, BD], F32R)
    wr_ap = dram_in("wr", [BD, D], F32R)
    bk_ap = dram_in("bkr", [R, D], F32)             # bk tiled over partitions
    bv_ap = dram_in("bvr", [R, BD], F32)
    br_ap = dram_in("brr", [R, D], F32)

    awout_ap = nc.dram_tensor("awout", [4, B, NQ * R], F32,
                              kind="ExternalOutput").ap()
    attout_ap = nc.dram_tensor("attout", [2, 128, D], F32,
                               kind="ExternalOutput").ap()

    with tile.TileContext(nc) as tc:
        with ExitStack() as ctx:
            per = ctx.enter_context(tc.tile_pool(name="persist", bufs=1))
            pre = ctx.enter_context(tc.tile_pool(name="prelude", bufs=2))
            wqp = ctx.enter_context(tc.tile_pool(name="wqpool", bufs=4))
            scrp = ctx.enter_context(tc.tile_pool(name="scrpool", bufs=2))
            qpsp = ctx.enter_context(tc.tile_pool(name="qps", bufs=4, space="PSUM"))
            aux = ctx.enter_context(tc.tile_pool(name="aux", bufs=2, space="PSUM"))
            aux1 = ctx.enter_context(tc.tile_pool(name="aux1", bufs=1, space="PSUM"))
            bqkp = ctx.enter_context(tc.tile_pool(name="bqkp", bufs=1, space="PSUM"))

            # ---- constants / small inputs ----
            # qt and kts carry 16 columns of zero padding so the 32-wide
            # stationary slices below stay in bounds for the last s / t;
            # the extra output rows land in otherwise-unused psum gap rows.
            qt = per.tile([D, SLOC * B + B], BF16)
            kvt = per.tile([D, B * R], F32R)
            wk = per.tile([D, D], F32R)
            bqt = per.tile([D, R * SLOC], BF16)
            wv = per.tile([D, BD], F32R)
            wr = per.tile([BD, D], F32R)
            bk_rep = per.tile([R, D], F32)
            bv_rep = per.tile([R, BD], F32)
            br_rep = per.tile([R, D], F32)
            ident = per.tile([128, 128], F32)
            # critical-path inputs on the scalar (HWDGE) ring so the sync
            # ring can start streaming wq chunk 0 at t=0
            nc.gpsimd.memset(qt[:, SLOC * B:], 0.0)
            for t, ap in ((kvt, kvt_ap), (wk, wk_ap), (bqt, bqt_ap)):
                nc.scalar.dma_start(t[:], ap[:])
            nc.scalar.dma_start(qt[:, :SLOC * B], qt_ap[:])
            # the rest on the gpsimd (SWDGE) queue
            for t, ap in ((wv, wv_ap), (wr, wr_ap), (bk_rep, bk_ap),
                          (bv_rep, bv_ap), (br_rep, br_ap)):
                nc.gpsimd.dma_start(t[:], ap[:])
            make_identity(nc, ident[:])

            krep = per.tile([128, R * D], BF16)     # K[b] rows, period 16
            kts = per.tile([D, R * B + B], BF16)    # K^T as [d, (t,b)], padded
            nc.gpsimd.memset(kts[:, R * B:], 0.0)
            vn = per.tile([R, B * BD], F32R)        # V_norm[b] as [t, (b,dd)]
            bqk = per.tile([128, NQ * R], F32)      # bias scores, aw layout

            # ---- K = kv@Wk + bk  (rows t), K^T, krep ----
            for b in range(B):
                lhs = kvt[:, b * R:(b + 1) * R]     # [i, t] for this b
                kb_ps = aux.tile([128, D], F32, tag="auxA")
                nc.tensor.matmul(kb_ps[:], lhs, wk[:], start=True, stop=True)
                kb = pre.tile([R, D], F32, tag="kb")
                nc.vector.tensor_add(kb[:], kb_ps[:], bk_rep[:])
                kbf = pre.tile([R, D], BF16, tag="kbf")
                nc.vector.tensor_copy(kbf[:], kb[:])
                nc.gpsimd.dma_start(
                    krep[b:b + 1, :].rearrange("o (t d) -> o t d", t=R), kbf[:])
                # K^T columns (t, b): strided DVE copy from a PE transpose
                tr_ps = aux.tile([128, D], F32, tag="auxA")
                nc.tensor.transpose(tr_ps[:], kb[:], ident[:])
                nc.vector.tensor_copy(
                    kts[:, :R * B].rearrange("p (t b) -> p t b", b=B)[:, :, b],
                    tr_ps[:])
            # replicate krep rows 0:16 over all 8 groups (doubling)
            nc.gpsimd.dma_start(krep[16:32, :], krep[0:16, :])
            nc.gpsimd.dma_start(krep[32:64, :], krep[0:32, :])
            nc.gpsimd.dma_start(krep[64:128, :], krep[0:64, :])

            # ---- bqK[32j+b, (t,q)] = bq[4q+j,t,:].K[b,t,:] ----
            bqk_ps = bqkp.tile([128, NQ * R], F32, tag="bqk")
            for t in range(R):
                kt_sl = kts[:, t * B:t * B + 2 * B]  # [d, 32] stationary
                for j in range(4):
                    nc.tensor.matmul(
                        bqk_ps[32 * j:32 * (j + 1), t * NQ:(t + 1) * NQ],
                        kt_sl, bqt[:, t * SLOC + j * NQ:t * SLOC + (j + 1) * NQ],
                        start=True, stop=True, tile_position=(0, 32 * j))
            nc.vector.tensor_copy(
                bqk[:].rearrange("p (q t) -> p t q", q=NQ),
                bqk_ps[:].rearrange("p (t q) -> p t q", q=NQ))

            # ---- V_norm from kv@Wv + bv ----
            for b in range(B):
                lhs = kvt[:, b * R:(b + 1) * R]
                vb_ps = aux.tile([128, BD], F32, tag="auxA")
                nc.tensor.matmul(vb_ps[:], lhs, wv[:], start=True, stop=True)
                vsb = pre.tile([R, BD], F32, tag="vsb")
                nc.vector.tensor_add(vsb[:], vb_ps[:], bv_rep[:])
                vsq = pre.tile([R, BD], F32, tag="vsq")
                ss = pre.tile([R, 1], F32, tag="ss")
                nc.scalar.activation(vsq[:], vsb[:], AF.Square, accum_out=ss[:])
                nrm = pre.tile([R, 1], F32, tag="nrm")
                nc.scalar.activation(nrm[:], ss[:], AF.Sqrt)
                nc.vector.tensor_scalar_max(nrm[:], nrm[:], 1e-12)
                vri = pre.tile([R, 1], F32, tag="vri")
                nc.vector.reciprocal(vri[:], nrm[:])
                nc.vector.tensor_scalar_mul(vn[:, b * BD:(b + 1) * BD], vsb[:], vri[:])

            aw_raw = per.tile([128, NQ * R], F32)   # row 32j+b, col q*128+t

            # ---- main loop: stream Wq, Q-projection + fused scores ----
            for tcx in range(NTC):
                wt = wqp.tile([D, SLOC * CHF], BF16, tag="wqt")
                eng = nc.sync if tcx % 2 == 0 else nc.scalar
                eng.dma_start(wt[:], wq_ap[tcx])
                for q in range(NQ):
                    for g in range(GP):
                        qps = qpsp.tile([128, 512], F32, tag="qps")
                        for j in range(4):
                            s = 4 * q + j
                            nc.tensor.matmul(
                                qps[32 * j:32 * (j + 1), :],
                                qt[:, s * B:s * B + 2 * B],
                                wt[:, s * CHF + g * 512:s * CHF + (g + 1) * 512],
                                start=True, stop=True,
                                tile_position=(0, 32 * j))
                        t0 = tcx * TCH + g * 4
                        col = q * R + t0
                        tmp = scrp.tile([128, 512], F32, tag="tmp")
                        nc.vector.tensor_mul(tmp[:], qps[:],
                                             krep[:, t0 * D:(t0 + 4) * D])
                        nc.vector.tensor_reduce(
                            aw_raw[:, col:col + 4],
                            tmp[:].rearrange("p (t d) -> p t d", t=4),
                            axis=AXL.X, op=ALU.add)

            # ---- postlude: relu, normalize, attended, output proj ----
            aw_norm = per.tile([128, NQ * R], F32)
            ssum = per.tile([128, NQ], F32)
            rden = per.tile([128, NQ], F32)
            rinv = per.tile([128, NQ], F32)
            awt = per.tile([128, NQ * R], F32R)

            nc.vector.tensor_add(aw_raw[:], aw_raw[:], bqk[:])
            for q in range(NQ):
                nc.scalar.activation(aw_norm[:, q * R:(q + 1) * R],
                                     aw_raw[:, q * R:(q + 1) * R], AF.Relu,
                                     accum_out=ssum[:, q:q + 1])
            nc.vector.tensor_scalar_add(rden[:], ssum[:], 1e-8)
            nc.vector.reciprocal(rinv[:], rden[:])
            for q in range(NQ):
                nc.scalar.activation(aw_norm[:, q * R:(q + 1) * R],
                                     aw_norm[:, q * R:(q + 1) * R], AF.Copy,
                                     scale=rinv[:, q:q + 1])
            for j in range(4):
                nc.gpsimd.dma_start(awout_ap[j], aw_norm[32 * j:32 * j + B, :])
            for q in range(NQ):
                trp = aux.tile([128, 128], F32, tag="auxA")
                nc.tensor.transpose(trp[:], aw_norm[:, q * R:(q + 1) * R], ident[:])
                nc.vector.tensor_copy(awt[:, q * R:(q + 1) * R], trp[:])

            attps = aux1.tile([BD, B * SLOC], F32, tag="auxC")
            awt4 = awt[:].rearrange("p (q j c) -> p q j c", q=NQ, j=4, c=32)
            for b in range(B):
                nc.tensor.matmul(attps[:, b * SLOC:(b + 1) * SLOC],
                                 vn[:, b * BD:(b + 1) * BD], awt4[:, :, :, b],
                                 start=True, stop=True)
            atts = per.tile([BD, B * SLOC], F32R)
            nc.vector.tensor_copy(atts[:], attps[:])

            for h in range(2):
                finps = aux.tile([128, D], F32, tag="auxA")
                nc.tensor.matmul(finps[:], atts[:, h * 128:(h + 1) * 128], wr[:],
                                 start=True, stop=True)
                fin = per.tile([128, D], F32, tag=f"fin{h}")
                nc.vector.tensor_add(fin[:], finps[:], br_rep[:])
                nc.gpsimd.dma_start(attout_ap[h], fin[:])

    nc.compile()
    return nc


def _get_nc():
    if "nc" not in _CACHE:
        _CACHE["nc"] = _build()
    return _CACHE["nc"]


def _make_in_maps(q_embed, kv_embed, Wk, bk, Wv, bv, Wq, bq, Wr, br):
    shared = {
        "kvt": np.ascontiguousarray(
            kv_embed.transpose(2, 0, 1).reshape(D, B * R)),
        "wk": np.ascontiguousarray(Wk),
        "wv": np.ascontiguousarray(Wv),
        "wr": np.ascontiguousarray(Wr),
        "bkr": np.ascontiguousarray(np.broadcast_to(bk, (R, D))),
        "bvr": np.ascontiguousarray(np.broadcast_to(bv, (R, BD))),
        "brr": np.ascontiguousarray(np.broadcast_to(br, (R, D))),
    }
    in_maps = []
    for c in range(NCORES):
        s0 = SLOC * c
        m = dict(shared)
        # [s, t, i, d] -> [tc, i, s, t8, d] with t = 8tc+t8
        wq_c = Wq[s0:s0 + SLOC].reshape(SLOC, NTC, TCH, D, D)
        m["wq"] = np.ascontiguousarray(
            wq_c.transpose(1, 3, 0, 2, 4)).reshape(
                NTC, D, SLOC * CHF).astype(NPBF16)
        m["qt"] = np.ascontiguousarray(
            q_embed[:, s0:s0 + SLOC, :].transpose(2, 1, 0)).reshape(
                D, SLOC * B).astype(NPBF16)
        # [s, t, d] with s = 4q+j -> [d, (t, j, q)]
        bq_c = bq[s0:s0 + SLOC].reshape(NQ, 4, R, D)
        m["bqt"] = np.ascontiguousarray(
            bq_c.transpose(3, 2, 1, 0)).reshape(D, R * SLOC).astype(NPBF16)
        in_maps.append(m)
    return in_maps


def _assemble(results):
    attended = np.empty((B, R, D), np.float32)
    aw = np.empty((B, R, R), np.float32)
    for c, r in enumerate(results):
        s0 = SLOC * c
        att = r["attout"].reshape(B, SLOC, D)
        attended[:, s0:s0 + SLOC, :] = att
        a = r["awout"].reshape(4, B, NQ, R).transpose(1, 2, 0, 3).reshape(B, SLOC, R)
        aw[:, s0:s0 + SLOC, :] = a
    return attended, aw


def _execute(inputs, **kwargs):
    nc = _get_nc()
    args = {k: np.asarray(inputs[k], np.float32) for k in
            ("q_embed", "kv_embed", "Wk", "bk", "Wv", "bv", "Wq", "bq",
             "Wr", "br")}
    in_maps = _make_in_maps(**args)
    res = run_bass_kernel_spmd(nc, in_maps, core_ids=list(range(NCORES)),
                               **kwargs)
    return _assemble(res.results), res


def kernel(**inputs):
    (attended, aw), _ = _execute(inputs)
    return attended, aw
